# revision 36
# baseline (speedup 1.0000x reference)
"""BinaryTreeGRU Trainium2 kernel (all-bf16 pipeline, v4).

Batch of B=64 complete binary trees (L=512 leaves, 1023 nodes each),
data-parallel over trees across 8 NeuronCores (8 trees/core).

Layout: feature-major "folded": activations in SBUF as
[128 partitions, 2 feature-blocks, n_cols] bf16.  Level h storage is
parity-split ([128, cb, colpar, n/2]) so the next level reads children
(left/right) as contiguous ranges.

Key engine facts driving the design (measured):
 - strided bf16 WRITES on DVE are ~10x slow; strided reads are ~1.4x.
 - ACT is stride-agnostic (cost = free size), so every tensor that must
   land parity-split is produced by a single-input ACT op (tanh / copy)
   writing the split layout directly.
 - matmuls/PSUM use 256-col sub-chunks (PSUM bank budget); DVE/gates/
   Wgh-path use 512-col super-chunks (halves per-op overhead).

Per internal super-chunk (512 cols = 2 sub-chunks):
  PE : 64 rzh matmuls (bf16, FWL) + 4 Wgh matmuls (N=512)
  ACT: 4 sigmoid pair-instrs, tanh->g split, copy(zs*-.5+1)->tt split
  DVE: rp=r*ch, s=rp_l+rp_r, zz=z*ch, zh=zz_l+zz_r, zs=z_l+z_r,
       v=tt*g (packed split), h=v+zh (packed write, strided zh read)
"""

import os
from contextlib import ExitStack

import numpy as np
import ml_dtypes

import concourse.bass as bass
import concourse.mybir as mybir
import concourse.tile as tile
from concourse import bacc
from concourse.bass_utils import run_bass_kernel_spmd

F32 = mybir.dt.float32
BF16 = mybir.dt.bfloat16
MULT = mybir.AluOpType.mult
ADD = mybir.AluOpType.add
SIGMOID = mybir.ActivationFunctionType.Sigmoid
TANH = mybir.ActivationFunctionType.Tanh
COPY = mybir.ActivationFunctionType.Copy

MEM = 256
IN_DIM = 256
B = 64
L = 512
NCORES = 8
BLOC = B // NCORES            # trees per core
N0 = BLOC * L                 # leaf columns per core = 4096
NLEVELS = 10                  # 4096,2048,...,8 columns
NCOLS = [N0 >> l for l in range(NLEVELS)]
TOT = sum(NCOLS)              # 8184
OFFS = np.cumsum([0] + NCOLS).tolist()
SC = 512                      # super-chunk (DVE / Wgh / gates unit)
SUB = 256                     # matmul / PSUM sub-chunk

LAST_RESULT = {}


def _wavefront_order(nchunks, d=2):
    """Topological super-chunk order interleaving levels."""
    pos = {}
    order = []
    remaining = [(lv, ci) for lv in range(NLEVELS)
                 for ci in range(nchunks[lv])]

    def parents(lv, ci):
        if lv == 0:
            return []
        if nchunks[lv - 1] == 2 * nchunks[lv]:
            return [(lv - 1, 2 * ci), (lv - 1, 2 * ci + 1)]
        return [(lv - 1, pc) for pc in range(nchunks[lv - 1])]

    t = 0
    while remaining:
        ready = []
        for (lv, ci) in remaining:
            ps = parents(lv, ci)
            dd = d if lv > 1 else (d if lv == 1 else 0)
            if all(p in pos and pos[p] + (2 if lv == 1 else dd + 1) <= t
                   for p in ps):
                ready.append((lv, ci))
        if ready:
            ch = max(ready, key=lambda c: (c[0], -c[1]))
            pos[ch] = t
            order.append(ch)
            remaining.remove(ch)
        else:
            order.append(None)   # spacing step (emit only a back)
        t += 1
    return order


def build_nc(fast_bias: bool):
    nc = bacc.Bacc("TRN2", target_bir_lowering=False, debug=False)

    d_x = nc.dram_tensor("xT", [128, 2, N0], BF16, kind="ExternalInput")
    d_wrzh = nc.dram_tensor("wrzh", [4, 128, 1024], BF16, kind="ExternalInput")
    d_wgrzx = nc.dram_tensor("wgrzx", [2, 128, 768], BF16, kind="ExternalInput")
    d_wgh = nc.dram_tensor("wgh", [2, 128, 256], BF16, kind="ExternalInput")
    d_bias = nc.dram_tensor("bias6", [6, 128, 1], F32, kind="ExternalInput")
    d_out = nc.dram_tensor("out", [2, 128, TOT], BF16, kind="ExternalOutput")

    x = d_x.ap()
    wrzh = d_wrzh.ap()
    wgrzx = d_wgrzx.ap()
    wgh = d_wgh.ap()
    bias6 = d_bias.ap()
    out = d_out.ap()

    mm = nc.tensor.matmul
    nsup = [max(1, NCOLS[lv] // SC) for lv in range(NLEVELS)]

    with tile.TileContext(nc) as tc, ExitStack() as ctx:
        singles = ctx.enter_context(tc.tile_pool(name="singles", bufs=1))
        xpool = ctx.enter_context(tc.tile_pool(name="xpool", bufs=6))
        gates_pool = ctx.enter_context(tc.tile_pool(name="gates", bufs=2))
        spool = ctx.enter_context(tc.tile_pool(name="spool", bufs=4))
        scratch = ctx.enter_context(tc.tile_pool(name="scratch", bufs=2))
        psum = ctx.enter_context(tc.tile_pool(name="psum", bufs=3, space="PSUM"))
        psumg = ctx.enter_context(tc.tile_pool(name="psumg", bufs=1, space="PSUM"))

        # --- initial loads split across queues: leaf needs wgrzx + x0 ---
        w_grzx = []
        for kc, eng in ((0, nc.sync), (1, nc.gpsimd)):
            t = singles.tile([128, 768], BF16, tag=f"wgrzx{kc}", name=f"wgrzx{kc}")
            eng.dma_start(out=t, in_=wgrzx[kc])
            w_grzx.append(t)
        b_t = []
        if not fast_bias:
            for i in range(6):
                t = singles.tile([128, 1], F32, tag=f"b{i}", name=f"b{i}")
                nc.sync.dma_start(out=t, in_=bias6[i])
                b_t.append(t)

        x_tiles = {}

        def load_x(ci, eng=None):
            # super-chunk granularity (512 cols)
            t = xpool.tile([128, 2, SC], BF16, tag="x", name=f"x{ci}")
            (eng or nc.gpsimd).dma_start(
                out=t, in_=x[:, :, ci * SC:(ci + 1) * SC])
            x_tiles[ci] = t

        # first x supers on idle queues so the first leaf matmul only
        # waits for the two wgrzx halves + x0, all in parallel
        load_x(0, nc.scalar)
        load_x(1, nc.sync)
        load_x(2, nc.gpsimd)

        w_rzh = []
        for kc in range(4):
            t = singles.tile([128, 1024], BF16, tag=f"wrzh{kc}", name=f"wrzh{kc}")
            (nc.sync if kc % 2 == 0 else nc.gpsimd).dma_start(
                out=t, in_=wrzh[kc])
            w_rzh.append(t)
        w_gh = []
        for kc in range(2):
            t = singles.tile([128, 256], BF16, tag=f"wgh{kc}", name=f"wgh{kc}")
            nc.sync.dma_start(out=t, in_=wgh[kc])
            w_gh.append(t)

        h_t = [singles.tile([128, 2, 2, max(1, NCOLS[l] // 2)], BF16,
                            tag=f"h{l}", name=f"h{l}", bufs=1)
               for l in range(NLEVELS)]

        def split_in_view(t, n, n2):
            """View of natural tile t [128, 2, n] iterated (cb, colpar, j2):
            strided (stride-2) read used by ops whose out is parity-split."""
            return bass.AP(tensor=t.tensor, offset=t.offset,
                           ap=[list(t.ap[0]), [n, 2], [1, 2], [2, n2]])

        state = {}   # (lv, ci) -> dict for the back phase

        def emit_leaf_front(ci):
            ns = SC
            c0 = ci * ns
            if ci + 3 < nsup[0]:
                load_x(ci + 3)
            # gz [128, cb, lr, ns]
            gz = gates_pool.tile([128, 2, 2, ns], BF16, tag="gz", name="gz")
            psg = psumg.tile([128, 2, ns], F32, tag="G", name="ps_gx")
            x_c = x_tiles.pop(ci)
            for lr in range(2):
                psq = psum.tile([128, 2, ns], F32, tag="P", name=f"psz{lr}")
                for cb in range(2):
                    col = 256 + lr * 256 + cb * 128
                    for kc in range(2):
                        mm(psq[:, cb, :], w_grzx[kc][:, col:col + 128],
                           x_c[:, kc, :], start=(kc == 0), stop=(kc == 1))
                gview = bass.AP(
                    tensor=gz.tensor, offset=gz.offset + lr * ns,
                    ap=[list(gz.ap[0]), [2 * ns, 2], [1, ns]])
                if fast_bias:
                    nc.scalar.activation(gview, psq, SIGMOID, bias=1.0)
                else:
                    for cb in range(2):
                        nc.scalar.activation(
                            gz[:, cb, lr, :], psq[:, cb, :],
                            SIGMOID, bias=b_t[2 + 2 * lr + cb])
            for cb in range(2):
                for kc in range(2):
                    mm(psg[:, cb, :],
                       w_grzx[kc][:, cb * 128:cb * 128 + 128],
                       x_c[:, kc, :], start=(kc == 0), stop=(kc == 1))
            n2 = ns // 2
            # tanh -> tg split [128, cb, colpar, j2]
            tg = scratch.tile([128, 2, 2, n2], BF16, tag="tg", name="tg")
            if fast_bias:
                nc.scalar.activation(tg, split_in_view(psg, ns, n2),
                                     TANH, bias=0.0)
            else:
                for cb in range(2):
                    nc.scalar.activation(
                        tg[:, cb],
                        bass.AP(tensor=psg.tensor,
                                offset=psg.offset + cb * ns,
                                ap=[list(psg.ap[0]), [1, 2], [2, n2]]),
                        TANH, bias=b_t[cb])
            zs = scratch.tile([128, 2, ns], BF16, tag="zs", name="zs")
            nc.vector.tensor_add(zs, gz[:, :, 0, :], gz[:, :, 1, :])
            tt = scratch.tile([128, 2, ns], BF16, tag="tt", name="tt")
            nc.vector.tensor_scalar(tt, zs, -0.5, 1.0, MULT, ADD)
            # h = tt * tg  (packed split write, strided tt read)
            j0 = c0 // 2
            nc.vector.tensor_mul(h_t[0][:, :, :, j0:j0 + n2],
                                 split_in_view(tt, ns, n2), tg)
            if ci == nsup[0] - 1:
                for cb in range(2):
                    nc.sync.dma_start(out=out[cb, :, OFFS[0]:OFFS[1]],
                                      in_=h_t[0][:, cb, :, :])

        def emit_front(lv, ci):
            if lv == 0:
                emit_leaf_front(ci)
                return
            ns = min(NCOLS[lv], SC)
            c0 = ci * ns
            nsub = max(1, ns // SUB)
            nn = min(ns, SUB)
            hp = h_t[lv - 1]

            # gates G [128, rz, cb, lr, ns]
            G = gates_pool.tile([128, 2, 2, 2, ns], BF16, tag="G", name="G")
            if ns <= 128:
                # small level: both gate pairs in one PSUM tile, one sigmoid
                ps = psum.tile([128, 2, 2, 2, ns], F32, tag="P", name="psT")
                for rz in range(2):
                    for lr in range(2):
                        for cb in range(2):
                            col = rz * 512 + lr * 256 + cb * 128
                            for kc in range(4):
                                mm(ps[:, rz, lr, cb, :],
                                   w_rzh[kc][:, col:col + 128],
                                   hp[:, kc % 2, kc // 2, c0:c0 + ns],
                                   start=(kc == 0), stop=(kc == 3))
                if fast_bias:
                    for rz in range(2):
                        gview = bass.AP(
                            tensor=G.tensor, offset=G.offset + rz * 4 * ns,
                            ap=[list(G.ap[0]), [ns, 2], [2 * ns, 2], [1, ns]])
                        nc.scalar.activation(gview, ps[:, rz], SIGMOID,
                                             bias=1.0)
                else:
                    for rz in range(2):
                        for lr in range(2):
                            for cb in range(2):
                                nc.scalar.activation(
                                    G[:, rz, cb, lr, :],
                                    ps[:, rz, lr, cb, :],
                                    SIGMOID, bias=b_t[2 + 2 * lr + cb])
            else:
                # quarter-granular PSUM [128, 2, ns]: N=ns(<=512) moving
                # matmuls -> half the MM/LDWEIGHTS count vs 256-col subs,
                # same sigmoid shapes
                for rz in range(2):
                    for lr in range(2):
                        psq = psum.tile([128, 2, ns], F32, tag="P",
                                        name=f"q{rz}{lr}")
                        for cb in range(2):
                            col = rz * 512 + lr * 256 + cb * 128
                            for kc in range(4):
                                mm(psq[:, cb, :],
                                   w_rzh[kc][:, col:col + 128],
                                   hp[:, kc % 2, kc // 2, c0:c0 + ns],
                                   start=(kc == 0), stop=(kc == 3))
                        gview = bass.AP(
                            tensor=G.tensor,
                            offset=G.offset + rz * 4 * ns + lr * ns,
                            ap=[list(G.ap[0]), [2 * ns, 2], [1, ns]])
                        if fast_bias:
                            nc.scalar.activation(gview, psq, SIGMOID,
                                                 bias=1.0)
                        else:
                            for cb in range(2):
                                nc.scalar.activation(
                                    G[:, rz, cb, lr, :], psq[:, cb, :],
                                    SIGMOID, bias=b_t[2 + 2 * lr + cb])

            hps = hp[:, :, :, c0:c0 + ns]
            rp = scratch.tile([128, 2, 2, ns], BF16, tag="rp", name="rp")
            nc.vector.tensor_mul(rp, G[:, 0], hps)
            s = spool.tile([128, 2, ns], BF16, tag="s", name="s")
            nc.vector.tensor_add(s, rp[:, :, 0, :], rp[:, :, 1, :])
            zz = scratch.tile([128, 2, 2, ns], BF16, tag="zz", name="zz")
            nc.vector.tensor_mul(zz, G[:, 1], hps)
            zh = spool.tile([128, 2, ns], BF16, tag="zh", name="zh")
            nc.vector.tensor_add(zh, zz[:, :, 0, :], zz[:, :, 1, :])
            zs = spool.tile([128, 2, ns], BF16, tag="zs", name="zs")
            nc.vector.tensor_add(zs, G[:, 1, :, 0, :], G[:, 1, :, 1, :])
            state[(lv, ci)] = dict(s=s, zh=zh, zs=zs, c0=c0, ncur=ns)

        def emit_back(lv, ci):
            if lv == 0:
                return
            st = state.pop((lv, ci))
            s, zh, zs = st["s"], st["zh"], st["zs"]
            c0, ns = st["c0"], st["ncur"]
            n2 = max(1, ns // 2)

            # tt = 1 - zs/2, parity-split, on ACT (stride-free); emitted
            # first so it's ready before tanh completes
            tt_s = scratch.tile([128, 2, 2, n2], BF16, tag="tt", name="tt_s")
            nc.scalar.activation(tt_s, split_in_view(zs, ns, n2),
                                 COPY, bias=1.0, scale=-0.5)
            psg = psumg.tile([128, 2, ns], F32, tag="G", name="ps_g")
            for mb in range(2):
                for kc in range(2):
                    mm(psg[:, mb, :], w_gh[kc][:, 128 * mb:128 * mb + 128],
                       s[:, kc, :], start=(kc == 0), stop=(kc == 1))
            # tanh -> g split; copy -> tt split (both stride-free on ACT)
            g_s = scratch.tile([128, 2, 2, n2], BF16, tag="gsb", name="g_s")
            if fast_bias:
                nc.scalar.activation(g_s, split_in_view(psg, ns, n2),
                                     TANH, bias=0.0)
            else:
                for cb in range(2):
                    nc.scalar.activation(
                        g_s[:, cb],
                        bass.AP(tensor=psg.tensor,
                                offset=psg.offset + cb * ns,
                                ap=[list(psg.ap[0]), [1, 2], [2, n2]]),
                        TANH, bias=b_t[cb])
            v_s = scratch.tile([128, 2, 2, n2], BF16, tag="v", name="v_s")
            nc.vector.tensor_mul(v_s, tt_s, g_s)
            # h = v + zh (packed split write, strided zh read)
            j0 = c0 // 2
            nc.vector.tensor_add(h_t[lv][:, :, :, j0:j0 + n2],
                                 v_s, split_in_view(zh, ns, n2))
            if ci == nsup[lv] - 1:
                for cb in range(2):
                    nc.sync.dma_start(out=out[cb, :, OFFS[lv]:OFFS[lv + 1]],
                                      in_=h_t[lv][:, cb, :])

        D = 2
        order = _wavefront_order(nsup, D)

        def parent_list(lv, ci):
            if lv == 0:
                return []
            if nsup[lv - 1] == 2 * nsup[lv]:
                return [(lv - 1, 2 * ci), (lv - 1, 2 * ci + 1)]
            return [(lv - 1, pc) for pc in range(nsup[lv - 1])]

        pending = []
        done = set()

        def pop_back():
            b = pending.pop(0)
            emit_back(*b)
            done.add(b)

        for ch in order:
            if ch is None:
                if pending:
                    pop_back()
                continue
            lv, ci = ch
            for par in parent_list(lv, ci):
                while par not in done:
                    pop_back()
            emit_front(lv, ci)
            pending.append(ch)
            while len(pending) > D:
                pop_back()
        while pending:
            pop_back()

    nc.compile()
    return nc


def _prep_inputs(inputs, Wgrzx, bgrzx, Wrzh, Wgh):
    """Host-side shard + layout prep. Returns (in_maps, fast_bias)."""
    x = np.ascontiguousarray(inputs, dtype=np.float32)
    Wgrzx = np.asarray(Wgrzx, dtype=np.float32)
    bgrzx = np.asarray(bgrzx, dtype=np.float32)
    Wrzh = np.asarray(Wrzh, dtype=np.float32)
    Wgh = np.asarray(Wgh, dtype=np.float32)

    fast_bias = bool(
        np.all(bgrzx[:MEM] == 0.0) and np.all(bgrzx[MEM:] == 1.0))

    bf = ml_dtypes.bfloat16
    wgrzxT = np.ascontiguousarray(Wgrzx.T.reshape(2, 128, 768)).astype(bf)
    wrzhT = np.ascontiguousarray(Wrzh.T.reshape(4, 128, 1024)).astype(bf)
    wghT = np.ascontiguousarray(Wgh.T.reshape(2, 128, 256)).astype(bf)
    bias6 = np.ascontiguousarray(bgrzx.reshape(6, 128, 1))

    in_maps = []
    for c in range(NCORES):
        xc = x[c * BLOC:(c + 1) * BLOC].reshape(N0, IN_DIM)
        xT = np.ascontiguousarray(
            xc.T.reshape(2, 128, N0).transpose(1, 0, 2)).astype(bf)
        in_maps.append({
            "xT": xT,
            "wrzh": wrzhT,
            "wgrzx": wgrzxT,
            "wgh": wghT,
            "bias6": bias6,
        })
    return in_maps, fast_bias


def _gather(results):
    """results: list of per-core {'out': [2,128,TOT]} -> [B, 2L-1, MEM]."""
    outs = []
    for c in range(len(results)):
        fm = np.asarray(results[c]["out"]).astype(np.float32).reshape(MEM, TOT)
        levels = []
        for lv in range(NLEVELS):
            n = NCOLS[lv]
            blk = fm[:, OFFS[lv]:OFFS[lv + 1]]
            nat = np.empty_like(blk)
            nat[:, 0::2] = blk[:, :n // 2]
            nat[:, 1::2] = blk[:, n // 2:]
            k = n // BLOC
            levels.append(nat.reshape(MEM, BLOC, k).transpose(1, 2, 0))
        outs.append(np.concatenate(levels, axis=1))
    return np.ascontiguousarray(
        np.concatenate(outs, axis=0), dtype=np.float32)


def kernel(**inputs):
    in_maps, fast_bias = _prep_inputs(
        inputs["inputs"], inputs["Wgrzx"], inputs["bgrzx"],
        inputs["Wrzh"], inputs["Wgh"])
    nc = build_nc(fast_bias)
    trace = bool(int(os.environ.get("BTGRU_TRACE", "0")))
    res = run_bass_kernel_spmd(
        nc, in_maps, core_ids=list(range(NCORES)), trace=trace)
    LAST_RESULT.clear()
    LAST_RESULT["exec_time_ns"] = res.exec_time_ns
    LAST_RESULT["profile_json"] = res.profile_json
    return _gather(res.results)


# revision 38
# speedup vs baseline: 1.1030x; 1.1030x over previous
"""BinaryTreeGRU Trainium2 kernel (all-bf16 pipeline, v4).

Batch of B=64 complete binary trees (L=512 leaves, 1023 nodes each),
data-parallel over trees across 8 NeuronCores (8 trees/core).

Layout: feature-major "folded": activations in SBUF as
[128 partitions, 2 feature-blocks, n_cols] bf16.  Level h storage is
parity-split ([128, cb, colpar, n/2]) so the next level reads children
(left/right) as contiguous ranges.

Key engine facts driving the design (measured):
 - strided bf16 WRITES on DVE are ~10x slow; strided reads are ~1.4x.
 - ACT is stride-agnostic (cost = free size), so every tensor that must
   land parity-split is produced by a single-input ACT op (tanh / copy)
   writing the split layout directly.
 - matmuls/PSUM use 256-col sub-chunks (PSUM bank budget); DVE/gates/
   Wgh-path use 512-col super-chunks (halves per-op overhead).

Per internal super-chunk (512 cols = 2 sub-chunks):
  PE : 64 rzh matmuls (bf16, FWL) + 4 Wgh matmuls (N=512)
  ACT: 4 sigmoid pair-instrs, tanh->g split, copy(zs*-.5+1)->tt split
  DVE: rp=r*ch, s=rp_l+rp_r, zz=z*ch, zh=zz_l+zz_r, zs=z_l+z_r,
       v=tt*g (packed split), h=v+zh (packed write, strided zh read)
"""

import os
from contextlib import ExitStack

import numpy as np
import ml_dtypes

import concourse.bass as bass
import concourse.mybir as mybir
import concourse.tile as tile
from concourse import bacc
from concourse.bass_utils import run_bass_kernel_spmd

F32 = mybir.dt.float32
BF16 = mybir.dt.bfloat16
MULT = mybir.AluOpType.mult
ADD = mybir.AluOpType.add
SIGMOID = mybir.ActivationFunctionType.Sigmoid
TANH = mybir.ActivationFunctionType.Tanh
COPY = mybir.ActivationFunctionType.Copy

MEM = 256
IN_DIM = 256
B = 64
L = 512
NCORES = 8
BLOC = B // NCORES            # trees per core
N0 = BLOC * L                 # leaf columns per core = 4096
NLEVELS = 10                  # 4096,2048,...,8 columns
NCOLS = [N0 >> l for l in range(NLEVELS)]
TOT = sum(NCOLS)              # 8184
OFFS = np.cumsum([0] + NCOLS).tolist()
SC = 512                      # super-chunk (DVE / Wgh / gates unit)
SUB = 256                     # matmul / PSUM sub-chunk

LAST_RESULT = {}


def _wavefront_order(nchunks, d=2):
    """Topological super-chunk order interleaving levels."""
    pos = {}
    order = []
    remaining = [(lv, ci) for lv in range(NLEVELS)
                 for ci in range(nchunks[lv])]

    def parents(lv, ci):
        if lv == 0:
            return []
        if nchunks[lv - 1] == 2 * nchunks[lv]:
            return [(lv - 1, 2 * ci), (lv - 1, 2 * ci + 1)]
        return [(lv - 1, pc) for pc in range(nchunks[lv - 1])]

    t = 0
    while remaining:
        ready = []
        for (lv, ci) in remaining:
            ps = parents(lv, ci)
            dd = d if lv > 1 else (d if lv == 1 else 0)
            if all(p in pos and pos[p] + (2 if lv == 1 else dd + 1) <= t
                   for p in ps):
                ready.append((lv, ci))
        if ready:
            ch = max(ready, key=lambda c: (c[0], -c[1]))
            pos[ch] = t
            order.append(ch)
            remaining.remove(ch)
        else:
            order.append(None)   # spacing step (emit only a back)
        t += 1
    return order


def build_nc(fast_bias: bool):
    nc = bacc.Bacc("TRN2", target_bir_lowering=False, debug=False)

    d_x = nc.dram_tensor("xT", [128, 2, N0], BF16, kind="ExternalInput")
    d_wrzh = nc.dram_tensor("wrzh", [4, 128, 1024], BF16, kind="ExternalInput")
    d_wgrzx = nc.dram_tensor("wgrzx", [2, 128, 768], BF16, kind="ExternalInput")
    d_wgh = nc.dram_tensor("wgh", [2, 128, 256], BF16, kind="ExternalInput")
    d_bias = nc.dram_tensor("bias6", [6, 128, 1], F32, kind="ExternalInput")
    d_out = nc.dram_tensor("out", [2, 128, TOT], BF16, kind="ExternalOutput")

    x = d_x.ap()
    wrzh = d_wrzh.ap()
    wgrzx = d_wgrzx.ap()
    wgh = d_wgh.ap()
    bias6 = d_bias.ap()
    out = d_out.ap()

    mm = nc.tensor.matmul
    nsup = [max(1, NCOLS[lv] // SC) for lv in range(NLEVELS)]

    with tile.TileContext(nc) as tc, ExitStack() as ctx:
        singles = ctx.enter_context(tc.tile_pool(name="singles", bufs=1))
        xpool = ctx.enter_context(tc.tile_pool(name="xpool", bufs=6))
        gates_pool = ctx.enter_context(tc.tile_pool(name="gates", bufs=2))
        spool = ctx.enter_context(tc.tile_pool(name="spool", bufs=4))
        scratch = ctx.enter_context(tc.tile_pool(name="scratch", bufs=2))
        psum = ctx.enter_context(tc.tile_pool(name="psum", bufs=3, space="PSUM"))
        psumg = ctx.enter_context(tc.tile_pool(name="psumg", bufs=1, space="PSUM"))

        # --- initial loads split across queues: leaf needs wgrzx + x0 ---
        w_grzx = []
        for kc, eng in ((0, nc.sync), (1, nc.gpsimd)):
            t = singles.tile([128, 768], BF16, tag=f"wgrzx{kc}", name=f"wgrzx{kc}")
            eng.dma_start(out=t, in_=wgrzx[kc])
            w_grzx.append(t)
        b_t = []
        if not fast_bias:
            for i in range(6):
                t = singles.tile([128, 1], F32, tag=f"b{i}", name=f"b{i}")
                nc.sync.dma_start(out=t, in_=bias6[i])
                b_t.append(t)

        x_tiles = {}

        def load_x(ci, eng=None):
            t = xpool.tile([128, 2, SUB], BF16, tag="x", name=f"x{ci}")
            (eng or nc.gpsimd).dma_start(
                out=t, in_=x[:, :, ci * SUB:(ci + 1) * SUB])
            x_tiles[ci] = t

        # first x chunks on the idle scalar queue so the first leaf matmul
        # only waits for the two wgrzx halves + x0, all in parallel
        load_x(0, nc.scalar)
        load_x(1, nc.sync)
        load_x(2, nc.gpsimd)
        load_x(3, nc.scalar)

        w_rzh = []
        for kc in range(4):
            t = singles.tile([128, 1024], BF16, tag=f"wrzh{kc}", name=f"wrzh{kc}")
            (nc.sync if kc % 2 == 0 else nc.gpsimd).dma_start(
                out=t, in_=wrzh[kc])
            w_rzh.append(t)
        w_gh = []
        for kc in range(2):
            t = singles.tile([128, 256], BF16, tag=f"wgh{kc}", name=f"wgh{kc}")
            nc.sync.dma_start(out=t, in_=wgh[kc])
            w_gh.append(t)

        h_t = [singles.tile([128, 2, 2, max(1, NCOLS[l] // 2)], BF16,
                            tag=f"h{l}", name=f"h{l}", bufs=1)
               for l in range(NLEVELS)]

        def split_in_view(t, n, n2):
            """View of natural tile t [128, 2, n] iterated (cb, colpar, j2):
            strided (stride-2) read used by ops whose out is parity-split."""
            return bass.AP(tensor=t.tensor, offset=t.offset,
                           ap=[list(t.ap[0]), [n, 2], [1, 2], [2, n2]])

        state = {}   # (lv, ci) -> dict for the back phase

        def emit_leaf_front(ci):
            ns = SC
            c0 = ci * ns
            nsub = ns // SUB
            for k in range(nsub):
                if 2 * ci + k + 4 < N0 // SUB:
                    load_x(2 * ci + k + 4)
            # gz [128, cb, lr, ns]
            gz = gates_pool.tile([128, 2, 2, ns], BF16, tag="gz", name="gz")
            psg = psumg.tile([128, 2, ns], F32, tag="G", name="ps_gx")
            for k in range(nsub):
                x_c = x_tiles.pop(2 * ci + k)
                ps = psum.tile([128, 2, 2, SUB], F32, tag="P", name="ps_z")
                for lr in range(2):
                    for cb in range(2):
                        col = 256 + lr * 256 + cb * 128
                        for kc in range(2):
                            mm(ps[:, lr, cb, :],
                               w_grzx[kc][:, col:col + 128],
                               x_c[:, kc, :], start=(kc == 0), stop=(kc == 1))
                # sigmoid pair -> gz[:, :, :, k*SUB:...] iterate (lr, cb, j)
                gview = bass.AP(
                    tensor=gz.tensor, offset=gz.offset + k * SUB,
                    ap=[list(gz.ap[0]), [ns, 2], [2 * ns, 2], [1, SUB]])
                if fast_bias:
                    nc.scalar.activation(gview, ps, SIGMOID, bias=1.0)
                else:
                    for lr in range(2):
                        for cb in range(2):
                            nc.scalar.activation(
                                gz[:, cb, lr, k * SUB:(k + 1) * SUB],
                                ps[:, lr, cb, :],
                                SIGMOID, bias=b_t[2 + 2 * lr + cb])
                for cb in range(2):
                    for kc in range(2):
                        mm(psg[:, cb, k * SUB:(k + 1) * SUB],
                           w_grzx[kc][:, cb * 128:cb * 128 + 128],
                           x_c[:, kc, :], start=(kc == 0), stop=(kc == 1))
            n2 = ns // 2
            # tanh -> tg split [128, cb, colpar, j2]
            tg = scratch.tile([128, 2, 2, n2], BF16, tag="tg", name="tg")
            if fast_bias:
                nc.scalar.activation(tg, split_in_view(psg, ns, n2),
                                     TANH, bias=0.0)
            else:
                for cb in range(2):
                    nc.scalar.activation(
                        tg[:, cb],
                        bass.AP(tensor=psg.tensor,
                                offset=psg.offset + cb * ns,
                                ap=[list(psg.ap[0]), [1, 2], [2, n2]]),
                        TANH, bias=b_t[cb])
            zs = scratch.tile([128, 2, ns], BF16, tag="zs", name="zs")
            nc.vector.tensor_add(zs, gz[:, :, 0, :], gz[:, :, 1, :])
            tt = scratch.tile([128, 2, ns], BF16, tag="tt", name="tt")
            nc.vector.tensor_scalar(tt, zs, -0.5, 1.0, MULT, ADD)
            # h = tt * tg  (packed split write, strided tt read)
            j0 = c0 // 2
            nc.vector.tensor_mul(h_t[0][:, :, :, j0:j0 + n2],
                                 split_in_view(tt, ns, n2), tg)
            if ci == nsup[0] - 1:
                for cb in range(2):
                    nc.sync.dma_start(out=out[cb, :, OFFS[0]:OFFS[1]],
                                      in_=h_t[0][:, cb, :, :])

        def emit_front(lv, ci):
            if lv == 0:
                emit_leaf_front(ci)
                return
            ns = min(NCOLS[lv], SC)
            c0 = ci * ns
            nsub = max(1, ns // SUB)
            nn = min(ns, SUB)
            hp = h_t[lv - 1]

            # gates G [128, rz, cb, lr, ns]
            G = gates_pool.tile([128, 2, 2, 2, ns], BF16, tag="G", name="G")
            if ns <= 128:
                # small level: both gate pairs in one PSUM tile, one sigmoid
                ps = psum.tile([128, 2, 2, 2, ns], F32, tag="P", name="psT")
                for rz in range(2):
                    for lr in range(2):
                        for cb in range(2):
                            col = rz * 512 + lr * 256 + cb * 128
                            for kc in range(4):
                                mm(ps[:, rz, lr, cb, :],
                                   w_rzh[kc][:, col:col + 128],
                                   hp[:, kc % 2, kc // 2, c0:c0 + ns],
                                   start=(kc == 0), stop=(kc == 3))
                if fast_bias:
                    for rz in range(2):
                        gview = bass.AP(
                            tensor=G.tensor, offset=G.offset + rz * 4 * ns,
                            ap=[list(G.ap[0]), [ns, 2], [2 * ns, 2], [1, ns]])
                        nc.scalar.activation(gview, ps[:, rz], SIGMOID,
                                             bias=1.0)
                else:
                    for rz in range(2):
                        for lr in range(2):
                            for cb in range(2):
                                nc.scalar.activation(
                                    G[:, rz, cb, lr, :],
                                    ps[:, rz, lr, cb, :],
                                    SIGMOID, bias=b_t[2 + 2 * lr + cb])
            else:
                for k in range(nsub):
                    cs = c0 + k * nn
                    for rz in range(2):
                        ps = psum.tile([128, 2, 2, nn], F32, tag="P",
                                       name=f"ps{rz}")
                        for lr in range(2):
                            for cb in range(2):
                                col = rz * 512 + lr * 256 + cb * 128
                                for kc in range(4):
                                    mm(ps[:, lr, cb, :],
                                       w_rzh[kc][:, col:col + 128],
                                       hp[:, kc % 2, kc // 2, cs:cs + nn],
                                       start=(kc == 0), stop=(kc == 3))
                        gview = bass.AP(
                            tensor=G.tensor,
                            offset=G.offset + rz * 4 * ns + k * nn,
                            ap=[list(G.ap[0]), [ns, 2], [2 * ns, 2], [1, nn]])
                        if fast_bias:
                            nc.scalar.activation(gview, ps, SIGMOID, bias=1.0)
                        else:
                            for lr in range(2):
                                for cb in range(2):
                                    nc.scalar.activation(
                                        G[:, rz, cb, lr, k * nn:(k + 1) * nn],
                                        ps[:, lr, cb, :],
                                        SIGMOID, bias=b_t[2 + 2 * lr + cb])

            hps = hp[:, :, :, c0:c0 + ns]
            rp = scratch.tile([128, 2, 2, ns], BF16, tag="rp", name="rp")
            nc.vector.tensor_mul(rp, G[:, 0], hps)
            s = spool.tile([128, 2, ns], BF16, tag="s", name="s")
            nc.vector.tensor_add(s, rp[:, :, 0, :], rp[:, :, 1, :])
            zz = scratch.tile([128, 2, 2, ns], BF16, tag="zz", name="zz")
            nc.vector.tensor_mul(zz, G[:, 1], hps)
            zh = spool.tile([128, 2, ns], BF16, tag="zh", name="zh")
            nc.vector.tensor_add(zh, zz[:, :, 0, :], zz[:, :, 1, :])
            zs = spool.tile([128, 2, ns], BF16, tag="zs", name="zs")
            nc.vector.tensor_add(zs, G[:, 1, :, 0, :], G[:, 1, :, 1, :])
            state[(lv, ci)] = dict(s=s, zh=zh, zs=zs, c0=c0, ncur=ns)

        def emit_back(lv, ci):
            if lv == 0:
                return
            st = state.pop((lv, ci))
            s, zh, zs = st["s"], st["zh"], st["zs"]
            c0, ns = st["c0"], st["ncur"]
            n2 = max(1, ns // 2)

            # tt = 1 - zs/2, parity-split, on ACT (stride-free); emitted
            # first so it's ready before tanh completes
            tt_s = scratch.tile([128, 2, 2, n2], BF16, tag="tt", name="tt_s")
            nc.scalar.activation(tt_s, split_in_view(zs, ns, n2),
                                 COPY, bias=1.0, scale=-0.5)
            g_s = scratch.tile([128, 2, 2, n2], BF16, tag="gsb", name="g_s")
            v_s = scratch.tile([128, 2, 2, n2], BF16, tag="v", name="v_s")
            j0 = c0 // 2
            # big backs run in two column halves so tanh(half0) overlaps
            # the PE's wgh(half1) and h completes earlier at boundaries
            nhalves = 2 if (ns >= 256 and fast_bias) else 1
            nh = ns // nhalves
            for hf in range(nhalves):
                h0 = hf * nh
                q2 = max(1, nh // 2)
                if hf == 0:
                    psg = psumg.tile([128, 2, nh], F32, tag="G", name="ps_g")
                else:
                    psg = psum.tile([128, 2, nh], F32, tag="P", name="ps_g2")
                for mb in range(2):
                    for kc in range(2):
                        mm(psg[:, mb, :],
                           w_gh[kc][:, 128 * mb:128 * mb + 128],
                           s[:, kc, h0:h0 + nh],
                           start=(kc == 0), stop=(kc == 1))
                jh = h0 // 2
                # tanh -> g split (stride-free on ACT)
                if fast_bias:
                    nc.scalar.activation(
                        g_s[:, :, :, jh:jh + q2],
                        bass.AP(tensor=psg.tensor, offset=psg.offset,
                                ap=[list(psg.ap[0]), [nh, 2], [1, 2],
                                    [2, q2]]),
                        TANH, bias=0.0)
                else:
                    for cb in range(2):
                        nc.scalar.activation(
                            g_s[:, cb],
                            bass.AP(tensor=psg.tensor,
                                    offset=psg.offset + cb * ns,
                                    ap=[list(psg.ap[0]), [1, 2], [2, n2]]),
                            TANH, bias=b_t[cb])
                nc.vector.tensor_mul(v_s[:, :, :, jh:jh + q2],
                                     tt_s[:, :, :, jh:jh + q2],
                                     g_s[:, :, :, jh:jh + q2])
                # h = v + zh (packed split write, strided zh read)
                nc.vector.tensor_add(
                    h_t[lv][:, :, :, j0 + jh:j0 + jh + q2],
                    v_s[:, :, :, jh:jh + q2],
                    bass.AP(tensor=zh.tensor, offset=zh.offset + h0,
                            ap=[list(zh.ap[0]), [ns, 2], [1, 2], [2, q2]]))
            if ci == nsup[lv] - 1:
                # tail-level stores on the idle scalar queue
                eng = nc.scalar if lv >= 4 else nc.sync
                for cb in range(2):
                    eng.dma_start(out=out[cb, :, OFFS[lv]:OFFS[lv + 1]],
                                  in_=h_t[lv][:, cb, :])

        D = 2
        order = _wavefront_order(nsup, D)

        def parent_list(lv, ci):
            if lv == 0:
                return []
            if nsup[lv - 1] == 2 * nsup[lv]:
                return [(lv - 1, 2 * ci), (lv - 1, 2 * ci + 1)]
            return [(lv - 1, pc) for pc in range(nsup[lv - 1])]

        pending = []
        done = set()

        def pop_back():
            b = pending.pop(0)
            emit_back(*b)
            done.add(b)

        for ch in order:
            if ch is None:
                if pending:
                    pop_back()
                continue
            lv, ci = ch
            for par in parent_list(lv, ci):
                while par not in done:
                    pop_back()
            emit_front(lv, ci)
            pending.append(ch)
            while len(pending) > D:
                pop_back()
        while pending:
            pop_back()

    nc.compile()
    return nc


def _prep_inputs(inputs, Wgrzx, bgrzx, Wrzh, Wgh):
    """Host-side shard + layout prep. Returns (in_maps, fast_bias)."""
    x = np.ascontiguousarray(inputs, dtype=np.float32)
    Wgrzx = np.asarray(Wgrzx, dtype=np.float32)
    bgrzx = np.asarray(bgrzx, dtype=np.float32)
    Wrzh = np.asarray(Wrzh, dtype=np.float32)
    Wgh = np.asarray(Wgh, dtype=np.float32)

    fast_bias = bool(
        np.all(bgrzx[:MEM] == 0.0) and np.all(bgrzx[MEM:] == 1.0))

    bf = ml_dtypes.bfloat16
    wgrzxT = np.ascontiguousarray(Wgrzx.T.reshape(2, 128, 768)).astype(bf)
    wrzhT = np.ascontiguousarray(Wrzh.T.reshape(4, 128, 1024)).astype(bf)
    wghT = np.ascontiguousarray(Wgh.T.reshape(2, 128, 256)).astype(bf)
    bias6 = np.ascontiguousarray(bgrzx.reshape(6, 128, 1))

    in_maps = []
    for c in range(NCORES):
        xc = x[c * BLOC:(c + 1) * BLOC].reshape(N0, IN_DIM)
        xT = np.ascontiguousarray(
            xc.T.reshape(2, 128, N0).transpose(1, 0, 2)).astype(bf)
        in_maps.append({
            "xT": xT,
            "wrzh": wrzhT,
            "wgrzx": wgrzxT,
            "wgh": wghT,
            "bias6": bias6,
        })
    return in_maps, fast_bias


def _gather(results):
    """results: list of per-core {'out': [2,128,TOT]} -> [B, 2L-1, MEM]."""
    outs = []
    for c in range(len(results)):
        fm = np.asarray(results[c]["out"]).astype(np.float32).reshape(MEM, TOT)
        levels = []
        for lv in range(NLEVELS):
            n = NCOLS[lv]
            blk = fm[:, OFFS[lv]:OFFS[lv + 1]]
            nat = np.empty_like(blk)
            nat[:, 0::2] = blk[:, :n // 2]
            nat[:, 1::2] = blk[:, n // 2:]
            k = n // BLOC
            levels.append(nat.reshape(MEM, BLOC, k).transpose(1, 2, 0))
        outs.append(np.concatenate(levels, axis=1))
    return np.ascontiguousarray(
        np.concatenate(outs, axis=0), dtype=np.float32)


def kernel(**inputs):
    in_maps, fast_bias = _prep_inputs(
        inputs["inputs"], inputs["Wgrzx"], inputs["bgrzx"],
        inputs["Wrzh"], inputs["Wgh"])
    nc = build_nc(fast_bias)
    trace = bool(int(os.environ.get("BTGRU_TRACE", "0")))
    res = run_bass_kernel_spmd(
        nc, in_maps, core_ids=list(range(NCORES)), trace=trace)
    LAST_RESULT.clear()
    LAST_RESULT["exec_time_ns"] = res.exec_time_ns
    LAST_RESULT["profile_json"] = res.profile_json
    return _gather(res.results)
